# revision 2
# baseline (speedup 1.0000x reference)
"""Trainium2 Bass kernel for the LSTM-VAE (nn_D_VAE_NET), 8 NeuronCores.

Strategy: hidden/tensor-parallel across the 8 cores.
  - Core c owns encoder-hidden units [128c,128c+128) and decoder-hidden units
    [128c,128c+128); it computes the 4x128 gate rows for those units.
  - Batch (256) is the matmul moving dimension (N=256) so float32r matmuls run
    at full PE rate; weights are the stationary operand and live in SBUF.
  - Per time step two AllGathers (h_enc slice, h_dec slice) rebuild the full
    hidden state on every core; mu/sigma/z and c_out are computed replicated.
  - Each core writes only its 32 batch columns of every output (selected with
    the partition_id register), and the host concatenates + transposes.

All activations/state are fp32; matmul inputs are float32r (same bits).
"""

import sys

sys.path.insert(0, "/opt/trn_rl_repo")
import numpy as np

import concourse.bass as bass
import concourse.mybir as mybir
import concourse.tile as tile
from concourse import bacc
from concourse import bass_utils
from concourse.bass import ds, ts

F32 = mybir.dt.float32
F32R = mybir.dt.float32r
AF = mybir.ActivationFunctionType
OP = mybir.AluOpType

NCORES = 8
T_FULL = 512
B, DIM, ZDIM, ENC, DEC = 256, 256, 128, 1024, 1024
HSL = ENC // NCORES  # 128 hidden units per core
BSL = B // NCORES    # 32 batch cols per core
P = 128


def _r(ap):
    return ap.bitcast(F32R)


def build_nc(T: int, null: bool = False):
    nc = bacc.Bacc("TRN2", target_bir_lowering=False, debug=False, num_devices=NCORES)

    xT = nc.dram_tensor("xT", [T, 2, P, B], F32R, kind="ExternalInput")
    wx = nc.dram_tensor("wx", [P, 2, 512], F32R, kind="ExternalInput")
    wxh = nc.dram_tensor("wxh", [P, 2, 512], F32R, kind="ExternalInput")
    whe = nc.dram_tensor("whe", [P, 8, 512], F32R, kind="ExternalInput")
    wzd = nc.dram_tensor("wzd", [P, 1, 512], F32R, kind="ExternalInput")
    whd = nc.dram_tensor("whd", [P, 8, 512], F32R, kind="ExternalInput")
    wmusig = nc.dram_tensor("wmusig", [P, 8, 256], F32R, kind="ExternalInput")
    wcout = nc.dram_tensor("wcout", [P, 8, 256], F32R, kind="ExternalInput")
    b_enc = nc.dram_tensor("b_enc", [P, 4], F32, kind="ExternalInput")
    b_dec = nc.dram_tensor("b_dec", [P, 4], F32, kind="ExternalInput")
    b_musig = nc.dram_tensor("b_musig", [P, 2], F32, kind="ExternalInput")
    b_cout = nc.dram_tensor("b_cout", [P, 2], F32, kind="ExternalInput")
    eT_d = nc.dram_tensor("eT", [P, B], F32, kind="ExternalInput")

    dec_o = nc.dram_tensor("dec_o", [T, P, 2, BSL], F32, kind="ExternalOutput")
    sig_o = nc.dram_tensor("sig_o", [T, P, BSL], F32, kind="ExternalOutput")
    mu_o = nc.dram_tensor("mu_o", [T, P, BSL], F32, kind="ExternalOutput")
    ls_o = nc.dram_tensor("ls_o", [T, P, BSL], F32, kind="ExternalOutput")
    z_o = nc.dram_tensor("z_o", [T, P, BSL], F32R, kind="ExternalOutput")

    with tile.TileContext(nc) as tc:
        with (
            tc.tile_pool(name="wpool", bufs=1) as wpool,
            tc.tile_pool(name="state", bufs=1) as state,
            tc.tile_pool(name="xp", bufs=3) as xp,
            tc.tile_pool(name="pw", bufs=3) as pw,
            tc.tile_pool(name="psum", bufs=8, space="PSUM") as psum,
            tc.tile_pool(name="ccd", bufs=3, space="DRAM") as ccd,
            tc.tile_pool(name="ccs", bufs=3, space="DRAM") as ccs,
        ):
            w_x = wpool.tile([P, 2, 512], F32R)
            w_xh = wpool.tile([P, 2, 512], F32R)
            w_he = wpool.tile([P, 8, 512], F32R)
            w_zd = wpool.tile([P, 1, 512], F32R)
            w_hd = wpool.tile([P, 8, 512], F32R)
            w_ms = wpool.tile([P, 8, 256], F32R)
            w_co = wpool.tile([P, 8, 256], F32R)
            bias_e = wpool.tile([P, 4], F32)
            bias_d = wpool.tile([P, 4], F32)
            bias_ms = wpool.tile([P, 2], F32)
            bias_co = wpool.tile([P, 2], F32)
            e_sb = wpool.tile([P, B], F32)
            for dst, src in [
                (w_x, wx), (w_xh, wxh), (w_he, whe), (w_zd, wzd), (w_hd, whd),
                (w_ms, wmusig), (w_co, wcout), (bias_e, b_enc), (bias_d, b_dec),
                (bias_ms, b_musig), (bias_co, b_cout), (e_sb, eT_d),
            ]:
                nc.sync.dma_start(dst[:], src[:])

            hE = [state.tile([P, 8, B], F32R, name=f"hE{i}") for i in range(2)]
            hD = [state.tile([P, 8, B], F32R, name=f"hD{i}") for i in range(2)]
            cE = state.tile([P, B], F32)
            cD = state.tile([P, B], F32)
            sigc = [state.tile([P, 2, B], F32, name=f"sigc{i}") for i in range(2)]
            for t0 in [cE, cD]:
                nc.vector.memset(t0[:], 0.0)
            for t0 in sigc:
                nc.vector.memset(t0[:], 0.5)
            for t0 in hE + hD:
                nc.vector.memset(t0[:].bitcast(F32), 0.0)

            pid = nc.sync.partition_id()
            pid2 = nc.scalar.partition_id()
            ob = ds(pid * BSL, BSL)
            ob2 = ds(pid2 * BSL, BSL)

            for t in range(T if not null else 0):
                cur, prv = t % 2, 1 - (t % 2)
                hEp, hEc = hE[prv], hE[cur]
                hDp, hDc = hD[prv], hD[cur]
                sgp, sgc = sigc[prv], sigc[cur]

                xt = xp.tile([P, 2, B], F32R, name="xt")
                nc.sync.dma_start(xt[:, 0], xT[t, 0])
                nc.sync.dma_start(xt[:, 1], xT[t, 1])
                xhat = xp.tile([P, 2, B], F32R, name="xhat")
                nc.vector.tensor_sub(xhat[:, 0], xt[:, 0], sgp[:, 0])
                nc.vector.tensor_sub(xhat[:, 1], xt[:, 1], sgp[:, 1])

                # encoder gates: x-part (early) -> hh -> xhat-part (late dep)
                ge = [psum.tile([P, B], F32, name=f"ge{m}", tag="ps") for m in range(4)]
                for m in range(4):
                    msl = ts(m, P)
                    for kt in range(2):
                        nc.tensor.matmul(ge[m][:], _r(w_x[:, kt, msl]), _r(xt[:, kt]),
                                         start=(kt == 0), stop=False)
                    for kt in range(8):
                        nc.tensor.matmul(ge[m][:], _r(w_he[:, kt, msl]), _r(hEp[:, kt]),
                                         start=False, stop=False)
                    for kt in range(2):
                        nc.tensor.matmul(ge[m][:], _r(w_xh[:, kt, msl]), _r(xhat[:, kt]),
                                         start=False, stop=(kt == 1))

                i_s = pw.tile([P, B], F32, name="i_s")
                f_s = pw.tile([P, B], F32, name="f_s")
                g_t = pw.tile([P, B], F32, name="g_t")
                o_s = pw.tile([P, B], F32, name="o_s")
                nc.scalar.activation(i_s[:], ge[0][:], AF.Sigmoid, bias=bias_e[:, 0:1])
                nc.scalar.activation(f_s[:], ge[1][:], AF.Sigmoid, bias=bias_e[:, 1:2])
                nc.scalar.activation(g_t[:], ge[2][:], AF.Tanh, bias=bias_e[:, 2:3])
                nc.scalar.activation(o_s[:], ge[3][:], AF.Sigmoid, bias=bias_e[:, 3:4])
                t1 = pw.tile([P, B], F32, name="t1")
                nc.vector.tensor_mul(t1[:], i_s[:], g_t[:])
                nc.vector.tensor_mul(cE[:], f_s[:], cE[:])
                nc.vector.tensor_add(cE[:], cE[:], t1[:])
                th = pw.tile([P, B], F32, name="th")
                nc.scalar.activation(th[:], cE[:], AF.Tanh)
                hnew_e = pw.tile([P, B], F32R, name="hnew_e")
                nc.vector.tensor_mul(hnew_e[:], o_s[:], th[:])

                cci_e = ccd.tile([P, B], F32R, name="cci_e")
                cco_e = ccs.tile([P * 8, B], F32R, name="cco_e", addr_space="Shared")
                nc.sync.dma_start(cci_e[:, 0:128], hnew_e[:, 0:128])
                nc.sync.dma_start(cci_e[:, 128:256], hnew_e[:, 128:256])
                nc.gpsimd.collective_compute(
                    "AllGather", OP.bypass,
                    replica_groups=[list(range(NCORES))],
                    ins=[cci_e.opt()], outs=[cco_e.opt()],
                )
                cco_e_r = cco_e.rearrange("(r p) b -> p r b", p=P)
                for r in range(8):
                    nc.sync.dma_start(hEc[:, r], cco_e_r[:, r])

                p_mu = psum.tile([P, B], F32, name="p_mu", tag="ps")
                p_ls = psum.tile([P, B], F32, name="p_ls", tag="ps")
                for kt in range(8):
                    nc.tensor.matmul(p_mu[:], _r(w_ms[:, kt, 0:128]), _r(hEc[:, kt]),
                                     start=(kt == 0), stop=(kt == 7))
                for kt in range(8):
                    nc.tensor.matmul(p_ls[:], _r(w_ms[:, kt, 128:256]), _r(hEc[:, kt]),
                                     start=(kt == 0), stop=(kt == 7))
                mu_sb = pw.tile([P, B], F32, name="mu_sb")
                ls_sb = pw.tile([P, B], F32, name="ls_sb")
                sg_sb = pw.tile([P, B], F32, name="sg_sb")
                z_sb = pw.tile([P, B], F32R, name="z_sb")
                nc.scalar.activation(mu_sb[:], p_mu[:], AF.Identity, bias=bias_ms[:, 0:1])
                nc.scalar.activation(ls_sb[:], p_ls[:], AF.Identity, bias=bias_ms[:, 1:2])
                nc.scalar.activation(sg_sb[:], ls_sb[:], AF.Exp)
                nc.vector.tensor_mul(z_sb[:], sg_sb[:], e_sb[:])
                nc.vector.tensor_add(z_sb[:], z_sb[:], mu_sb[:])

                # decoder gates: hh first (early dep), z-part last
                gd = [psum.tile([P, B], F32, name=f"gd{m}", tag="ps") for m in range(4)]
                for m in range(4):
                    msl = ts(m, P)
                    for kt in range(8):
                        nc.tensor.matmul(gd[m][:], _r(w_hd[:, kt, msl]), _r(hDp[:, kt]),
                                         start=(kt == 0), stop=False)
                    nc.tensor.matmul(gd[m][:], _r(w_zd[:, 0, msl]), _r(z_sb[:]),
                                     start=False, stop=True)

                i_d = pw.tile([P, B], F32, name="i_d")
                f_d = pw.tile([P, B], F32, name="f_d")
                g_d = pw.tile([P, B], F32, name="g_d")
                o_d = pw.tile([P, B], F32, name="o_d")
                nc.scalar.activation(i_d[:], gd[0][:], AF.Sigmoid, bias=bias_d[:, 0:1])
                nc.scalar.activation(f_d[:], gd[1][:], AF.Sigmoid, bias=bias_d[:, 1:2])
                nc.scalar.activation(g_d[:], gd[2][:], AF.Tanh, bias=bias_d[:, 2:3])
                nc.scalar.activation(o_d[:], gd[3][:], AF.Sigmoid, bias=bias_d[:, 3:4])
                t2 = pw.tile([P, B], F32, name="t2")
                nc.vector.tensor_mul(t2[:], i_d[:], g_d[:])
                nc.vector.tensor_mul(cD[:], f_d[:], cD[:])
                nc.vector.tensor_add(cD[:], cD[:], t2[:])
                th_d = pw.tile([P, B], F32, name="th_d")
                nc.scalar.activation(th_d[:], cD[:], AF.Tanh)
                hnew_d = pw.tile([P, B], F32R, name="hnew_d")
                nc.vector.tensor_mul(hnew_d[:], o_d[:], th_d[:])

                cci_d = ccd.tile([P, B], F32R, name="cci_d")
                cco_d = ccs.tile([P * 8, B], F32R, name="cco_d", addr_space="Shared")
                nc.sync.dma_start(cci_d[:, 0:128], hnew_d[:, 0:128])
                nc.sync.dma_start(cci_d[:, 128:256], hnew_d[:, 128:256])
                nc.gpsimd.collective_compute(
                    "AllGather", OP.bypass,
                    replica_groups=[list(range(NCORES))],
                    ins=[cci_d.opt()], outs=[cco_d.opt()],
                )
                cco_d_r = cco_d.rearrange("(r p) b -> p r b", p=P)
                for r in range(8):
                    nc.sync.dma_start(hDc[:, r], cco_d_r[:, r])

                pco = [psum.tile([P, B], F32, name=f"pco{m}", tag="ps") for m in range(2)]
                for m in range(2):
                    for kt in range(8):
                        nc.tensor.matmul(pco[m][:], _r(w_co[:, kt, ts(m, P)]),
                                         _r(hDc[:, kt]), start=(kt == 0), stop=(kt == 7))
                cout = pw.tile([P, 2, B], F32, name="cout")
                nc.scalar.activation(cout[:, 0], pco[0][:], AF.Sigmoid, bias=bias_co[:, 0:1])
                nc.scalar.activation(cout[:, 1], pco[1][:], AF.Sigmoid, bias=bias_co[:, 1:2])
                # reference applies sigmoid AGAIN to the carried c_out for x_hat
                nc.scalar.activation(sgc[:, 0], cout[:, 0], AF.Sigmoid)
                nc.scalar.activation(sgc[:, 1], cout[:, 1], AF.Sigmoid)

                nc.sync.dma_start(dec_o[t, :, 0], cout[:, 0, ob])
                nc.sync.dma_start(dec_o[t, :, 1], cout[:, 1, ob])
                nc.sync.dma_start(mu_o[t], mu_sb[:, ob])
                nc.scalar.dma_start(ls_o[t], ls_sb[:, ob2])
                nc.scalar.dma_start(sig_o[t], sg_sb[:, ob2])
                nc.scalar.dma_start(z_o[t], z_sb[:, ob2])

    nc.compile()
    return nc


def prep_inputs(inputs: dict, T: int) -> list[dict]:
    f32 = np.float32
    x = np.ascontiguousarray(np.asarray(inputs["input_data"], f32)[:, :T, :])
    xTd = np.ascontiguousarray(x.transpose(1, 2, 0)).reshape(T, 2, P, B)

    Wih = np.asarray(inputs["W_ih_enc"], f32)
    Whe = np.asarray(inputs["W_hh_enc"], f32)
    Wzd = np.asarray(inputs["W_ih_dec"], f32)
    Whd = np.asarray(inputs["W_hh_dec"], f32)
    wmu = np.asarray(inputs["w_mu"], f32)
    wsg = np.asarray(inputs["w_sigma"], f32)
    whdec = np.asarray(inputs["w_h_dec"], f32)
    be = np.asarray(inputs["b_ih_enc"], f32) + np.asarray(inputs["b_hh_enc"], f32)
    bd = np.asarray(inputs["b_ih_dec"], f32) + np.asarray(inputs["b_hh_dec"], f32)
    bmu = np.asarray(inputs["b_mu"], f32)
    bsg = np.asarray(inputs["b_sigma"], f32)
    bco = np.asarray(inputs["b_h_dec"], f32)
    eT = np.ascontiguousarray(np.asarray(inputs["e"], f32).T)

    def sb_w(Wsub):
        K = Wsub.shape[0]
        return np.ascontiguousarray(Wsub.reshape(K // P, P, -1).transpose(1, 0, 2))

    wms_full = np.concatenate([wmu, wsg], axis=1)

    in_maps = []
    for c in range(NCORES):
        rows = np.concatenate([np.arange(g * 1024 + c * HSL, g * 1024 + c * HSL + HSL)
                               for g in range(4)])
        in_maps.append({
            "xT": xTd,
            "wx": sb_w(Wih[rows, 0:256].T.copy()),
            "wxh": sb_w(Wih[rows, 256:512].T.copy()),
            "whe": sb_w(Whe[rows, :].T.copy()),
            "wzd": sb_w(Wzd[rows, :].T.copy()),
            "whd": sb_w(Whd[rows, :].T.copy()),
            "wmusig": sb_w(wms_full),
            "wcout": sb_w(whdec),
            "b_enc": np.ascontiguousarray(be[rows].reshape(4, P).T),
            "b_dec": np.ascontiguousarray(bd[rows].reshape(4, P).T),
            "b_musig": np.ascontiguousarray(np.stack([bmu, bsg], axis=1)),
            "b_cout": np.ascontiguousarray(bco.reshape(2, P).T),
            "eT": eT,
        })
    return in_maps


def assemble_outputs(results: list[dict], T: int):
    decs, sigs, mus, lss, zs = [], [], [], [], []
    for c in range(NCORES):
        r = results[c]
        decs.append(r["dec_o"].transpose(3, 0, 2, 1).reshape(BSL, T, DIM))
        sigs.append(r["sig_o"].transpose(2, 0, 1))
        mus.append(r["mu_o"].transpose(2, 0, 1))
        lss.append(r["ls_o"].transpose(2, 0, 1))
        zs.append(r["z_o"].transpose(2, 0, 1))
    return (
        np.ascontiguousarray(np.concatenate(decs, 0)),
        np.ascontiguousarray(np.concatenate(sigs, 0)),
        np.ascontiguousarray(np.concatenate(mus, 0)),
        np.ascontiguousarray(np.concatenate(lss, 0)),
        np.ascontiguousarray(np.concatenate(zs, 0)),
    )


_NC_CACHE: dict = {}


def _get_nc(T: int, null: bool = False):
    key = (T, null)
    if key not in _NC_CACHE:
        _NC_CACHE[key] = build_nc(T, null=null)
    return _NC_CACHE[key]


def kernel(**inputs):
    T = int(np.asarray(inputs["input_data"]).shape[1])
    nc = _get_nc(T)
    in_maps = prep_inputs(inputs, T)
    res = bass_utils.run_bass_kernel_spmd(nc, in_maps, core_ids=list(range(NCORES)))
    return assemble_outputs(res.results, T)
